# revision 26
# baseline (speedup 1.0000x reference)
"""AdditiveAttention kernel for 8 TRN2 NeuronCores.

reference:
    hidden  = tanh(keys + query[:,None,:] + weight_feedback)   [B,T,D]
    energies= einsum("btd,d->bt", hidden, v)                   [B,T]
    masked softmax over T (t < enc_seq_len[b])
    context = einsum("bt,btd->bd", weights, value)             [B,D]
    returns (context, weights[:,:,None])

Sharding: data-parallel over B across 8 cores (4 examples per core), no
cross-core communication. DMA-bound: 96 MiB/core must stream from HBM
(~283 us at ~360 GB/s); every compute engine is kept well under that.

Per-core device program, two phases per example b:
  phase A (energies), per 128-row t-subtile j (1MB k/wf DMA tiles on the
  SP HWDGE ring; all small loads + stores ride the ACT ring so SP only
  streams):
    PE  : s_psum = I@k + I@wf + ones_row@q   (fp32r identity-matmul adds,
          full PE rate for moving dim >= 256 — keeps the adds off the DVE)
    ACT : tanh_sb = tanh(s_psum)             (PSUM -> SBUF)
    DVE : e_col = reduce(tanh_sb * v_bcast)  (mul + reduce; TTR would fuse
          them but crashes the device)
    ACT : exp;  DVE: p_col = exp * mask_col
  between phases: S = sum(p) via ones-matmuls + reciprocal; weights
    w = p/S written out (fp32r) — all overlapping phase B's DMAs.
  phase B (context): per value tile, PE accumulates
    ctx_psum[2,D] += w_col.T @ value_tile    (normalized weights, so the
    final tail is only a chunked PSUM->SBUF copy + store).
"""

import numpy as np
from contextlib import ExitStack

B, T, D = 32, 2048, 1024
NCORES = 8
BPC = B // NCORES          # 4 examples per core
SUB = 128                  # t-subtile rows (partition dim)
NT = T // SUB              # 16 subtiles per example
NCH = D // 512             # 512-wide matmul chunks

_PROGRAM_CACHE = {}


def build_program(bpc=BPC, t=T, d=D, wide=1, kb=3, wb=3, vb_bufs=4,
                  two_phase=False, val_engine="sync", val_wide=None):
    import concourse.bacc as bacc
    import concourse.tile as tile
    import concourse.mybir as mybir

    f32 = mybir.dt.float32
    f32r = mybir.dt.float32r
    AF = mybir.ActivationFunctionType
    ALU = mybir.AluOpType
    AX = mybir.AxisListType

    nt = t // SUB
    nch = d // 512
    if val_wide is None:
        val_wide = wide
    assert nt % wide == 0 and nt % val_wide == 0

    nc = bacc.Bacc("TRN2", target_bir_lowering=False, debug=False,
                   num_devices=NCORES, enable_asserts=False)

    keys = nc.dram_tensor("keys", [bpc, t, d], f32r, kind="ExternalInput").ap()
    wf = nc.dram_tensor("wf", [bpc, t, d], f32r, kind="ExternalInput").ap()
    val = nc.dram_tensor("value", [bpc, t, d], f32r, kind="ExternalInput").ap()
    qry = nc.dram_tensor("query", [bpc, d], f32r, kind="ExternalInput").ap()
    vb = nc.dram_tensor("vb", [SUB, d], f32, kind="ExternalInput").ap()
    maskT = nc.dram_tensor("maskT", [bpc, SUB, nt], f32, kind="ExternalInput").ap()
    ident = nc.dram_tensor("ident", [SUB, SUB], f32r, kind="ExternalInput").ap()
    ones_row = nc.dram_tensor("ones_row", [1, SUB], f32r, kind="ExternalInput").ap()
    ones_col = nc.dram_tensor("ones_col", [SUB, 1], f32r, kind="ExternalInput").ap()

    ctx_out = nc.dram_tensor("ctx_out", [bpc, d], f32, kind="ExternalOutput").ap()
    w_out = nc.dram_tensor("w_out", [bpc, SUB, nt], f32r, kind="ExternalOutput").ap()

    with tile.TileContext(nc) as tc, ExitStack() as ctx:
        cpool = ctx.enter_context(tc.tile_pool(name="consts", bufs=1))
        kpool = ctx.enter_context(tc.tile_pool(name="kpool", bufs=kb))
        wpool = ctx.enter_context(tc.tile_pool(name="wpool", bufs=wb))
        vpool = ctx.enter_context(tc.tile_pool(name="vpool", bufs=vb_bufs))
        tpool = ctx.enter_context(tc.tile_pool(name="tpool", bufs=2))
        scrpool = ctx.enter_context(tc.tile_pool(name="scrpool", bufs=2))
        smpool = ctx.enter_context(tc.tile_pool(name="smpool", bufs=4))
        expool = ctx.enter_context(tc.tile_pool(name="expool", bufs=2))
        ps_s = ctx.enter_context(tc.tile_pool(name="ps_s", bufs=2, space="PSUM"))
        ps_ctx = ctx.enter_context(tc.tile_pool(name="ps_ctx", bufs=1, space="PSUM"))
        ps_sm = ctx.enter_context(tc.tile_pool(name="ps_sm", bufs=2, space="PSUM"))

        # ident/ones_row feed the very first matmuls — load them first.
        # vb (512KB) and ones_col are needed later; their DMAs are emitted
        # after the first big loads so the DMA engines start on 1MB tiles.
        vb_t = cpool.tile([SUB, d], f32)
        id_t = cpool.tile([SUB, SUB], f32r)
        nc.scalar.dma_start(out=id_t[:], in_=ident[:])
        onr_t = cpool.tile([1, SUB], f32r)
        nc.scalar.dma_start(out=onr_t[:], in_=ones_row[:])
        onc_t = cpool.tile([SUB, 1], f32r)
        nc.scalar.dma_start(out=vb_t[:], in_=vb[:])
        nc.scalar.dma_start(out=onc_t[:], in_=ones_col[:])

        id_r = id_t[:]
        onr_r = onr_t[:]
        onc_r = onc_t[:]

        for b in range(bpc):
            q_t = expool.tile([1, d], f32r, tag="q")
            nc.scalar.dma_start(out=q_t[:], in_=qry[b : b + 1, :])
            mk_t = expool.tile([SUB, nt], f32, tag="mk")
            nc.scalar.dma_start(out=mk_t[:], in_=maskT[b])
            p_t = expool.tile([SUB, nt], f32r, tag="p")
            # fp32r matmul needs even stationary-M and even dst-N: use a
            # 0-stride broadcast of the p column (M=2) and a [2, d] dst.
            ctx_ps = ps_ctx.tile([2, d], f32, tag="ctx")

            def wide_dma(pool, tag, src, jw, engine=nc.sync, w_=None):
                w_ = wide if w_ is None else w_
                tile_ = pool.tile([SUB, w_ * d], f32r, tag=tag, name=tag)
                rows = slice(jw * w_ * SUB, (jw + 1) * w_ * SUB)
                if w_ == 1:
                    engine.dma_start(out=tile_[:], in_=src[b, rows, :])
                else:
                    engine.dma_start(
                        out=tile_[:].rearrange("p (c d) -> p c d", c=w_),
                        in_=src[b, rows, :].rearrange("(c p) d -> p c d",
                                                      c=w_))
                return tile_

            def energies(jw):
                k_t = wide_dma(kpool, "k", keys, jw)
                wf_t = wide_dma(wpool, "wf", wf, jw)
                for u in range(wide):
                    j = jw * wide + u
                    ksub = k_t[:, u * d : (u + 1) * d]
                    wsub = wf_t[:, u * d : (u + 1) * d]

                    s_ps = ps_s.tile([SUB, d], f32, tag="s", name="s_ps")
                    for c in range(nch):
                        sl = slice(512 * c, 512 * (c + 1))
                        nc.tensor.matmul(s_ps[:, sl], id_r, ksub[:, sl],
                                         start=True, stop=False)
                        nc.tensor.matmul(s_ps[:, sl], id_r, wsub[:, sl],
                                         start=False, stop=False)
                        nc.tensor.matmul(s_ps[:, sl], onr_r, q_t[0:1, sl],
                                         start=False, stop=True)

                    th_t = tpool.tile([SUB, d], f32, tag="th", name="th_t")
                    nc.scalar.activation(th_t[:], s_ps[:], AF.Tanh)

                    # (tensor_tensor_reduce would fuse these, but it crashes
                    # the device — NRT_EXEC_UNIT_UNRECOVERABLE)
                    scr_t = scrpool.tile([SUB, d], f32, tag="scr", name="scr_t")
                    e_col = smpool.tile([SUB, 1], f32, tag="ecol", name="e_col")
                    nc.vector.tensor_mul(scr_t[:], th_t[:], vb_t[:])
                    nc.vector.tensor_reduce(e_col[:], scr_t[:], axis=AX.X,
                                            op=ALU.add)

                    pe_col = smpool.tile([SUB, 1], f32, tag="pecol",
                                         name="pe_col")
                    nc.scalar.activation(pe_col[:], e_col[:], AF.Exp)
                    nc.vector.tensor_mul(p_t[:, j : j + 1], pe_col[:],
                                         mk_t[:, j : j + 1])

            def context(jw, v_t, col_src, vw):
                for u in range(vw):
                    j = jw * vw + u
                    p_r = col_src[:, j : j + 1].broadcast_to((SUB, 2))
                    for c in range(nch):
                        nc.tensor.matmul(
                            ctx_ps[:, 512 * c : 512 * (c + 1)], p_r,
                            v_t[:, u * d + 512 * c : u * d + 512 * (c + 1)],
                            start=(j == 0), stop=(j == nt - 1),
                            skip_group_check=True)

            def denom_and_weights():
                """Softmax denominator + weights output. Depends only on p_t
                (end of the energies phase), so it overlaps the value phase.
                Output DMAs ride the ACT HWDGE ring (nc.scalar) so the SP
                ring stays free for input loads (rings are FIFO)."""
                rowsum = smpool.tile([SUB, 1], f32, tag="rowsum")
                nc.vector.tensor_reduce(rowsum[:], p_t[:], axis=AX.X, op=ALU.add)
                rowsum_r = smpool.tile([SUB, 2], f32r, tag="rowsumr")
                nc.vector.tensor_copy(rowsum_r[:],
                                      rowsum[:, 0:1].broadcast_to((SUB, 2)))

                s1_ps = ps_sm.tile([2, 2], f32, tag="sm")
                nc.tensor.matmul(s1_ps[:], onc_r.broadcast_to((SUB, 2)),
                                 rowsum_r[:])
                s1_sb = smpool.tile([1, 2], f32r, tag="s1sb")
                nc.vector.tensor_copy(s1_sb[:], s1_ps[0:1, :])

                sbb_ps = ps_sm.tile([SUB, 2], f32, tag="sm")
                nc.tensor.matmul(sbb_ps[:], onr_r, s1_sb[:])
                S_sb = smpool.tile([SUB, 1], f32, tag="Ssb")
                nc.vector.tensor_copy(S_sb[:], sbb_ps[:, 0:1])
                rS = smpool.tile([SUB, 1], f32, tag="rS")
                nc.vector.reciprocal(rS[:], S_sb[:])

                w_t = expool.tile([SUB, nt], f32r, tag="w")
                nc.vector.tensor_scalar_mul(w_t[:], p_t[:], rS[:])
                nc.scalar.dma_start(out=w_out[b], in_=w_t[:])
                return rS, w_t

            v_eng = nc.sync if val_engine == "sync" else nc.scalar
            ctx_sb = expool.tile([1, d], f32, tag="ctxsb")
            if two_phase:
                for jw in range(nt // wide):
                    energies(jw)
                rS, w_t = denom_and_weights()
                # ctx accumulates already-normalized weights, so after the
                # last value tile only a PSUM->SBUF copy + DMA remain. The
                # last example streams value at fine grain to shorten the
                # post-last-byte critical path.
                vw = val_wide
                for jw in range(nt // vw):
                    context(jw, wide_dma(vpool, "v", val, jw, engine=v_eng,
                                         w_=vw), w_t, vw)
                for c in range(nch):
                    sl = slice(512 * c, 512 * (c + 1))
                    nc.vector.tensor_copy(ctx_sb[:, sl], ctx_ps[0:1, sl])
                    nc.scalar.dma_start(out=ctx_out[b : b + 1, sl],
                                        in_=ctx_sb[:, sl])
            else:
                for jw in range(nt // wide):
                    energies(jw)
                    context(jw, wide_dma(vpool, "v", val, jw, engine=v_eng,
                                         w_=wide), p_t, wide)
                rS, w_t = denom_and_weights()
                nc.vector.tensor_scalar_mul(ctx_sb[:], ctx_ps[0:1, :],
                                            rS[0:1, :])
                nc.scalar.dma_start(out=ctx_out[b : b + 1, :], in_=ctx_sb[:])

    nc.compile()
    return nc


BEST = dict(wide=2, kb=5, wb=5, vb_bufs=4, two_phase=True)


def _get_program():
    key = (BPC, T, D)
    if key not in _PROGRAM_CACHE:
        _PROGRAM_CACHE[key] = build_program(*key, **BEST)
    return _PROGRAM_CACHE[key]


def make_in_maps(keys, value, query, weight_feedback, v, enc_seq_len):
    """Shard full inputs into per-core in_maps (+ host-precomputed consts)."""
    keys = np.ascontiguousarray(keys, dtype=np.float32)
    value = np.ascontiguousarray(value, dtype=np.float32)
    query = np.ascontiguousarray(query, dtype=np.float32)
    wf = np.ascontiguousarray(weight_feedback, dtype=np.float32)
    v = np.ascontiguousarray(v, dtype=np.float32)
    lens = np.asarray(enc_seq_len).astype(np.int64)

    vb = np.broadcast_to(v, (SUB, D)).copy()
    ident = np.eye(SUB, dtype=np.float32)
    ones_row = np.ones((1, SUB), dtype=np.float32)
    ones_col = np.ones((SUB, 1), dtype=np.float32)

    # maskT[b, p, j] = 1.0 if (j*SUB + p) < len[b]
    tidx = (np.arange(T).reshape(NT, SUB).T)[None, :, :]      # [1, SUB, NT]
    maskT = (tidx < lens[:, None, None]).astype(np.float32)    # [B, SUB, NT]

    in_maps = []
    for c in range(NCORES):
        s = slice(c * BPC, (c + 1) * BPC)
        in_maps.append({
            "keys": keys[s],
            "wf": wf[s],
            "value": value[s],
            "query": query[s],
            "vb": vb,
            "maskT": np.ascontiguousarray(maskT[s]),
            "ident": ident,
            "ones_row": ones_row,
            "ones_col": ones_col,
        })
    return in_maps


def assemble_outputs(results):
    """results: list (per core) of dicts with ctx_out [BPC,D], w_out [BPC,SUB,NT]."""
    context = np.concatenate([np.asarray(r["ctx_out"]) for r in results], axis=0)
    wT = np.concatenate([np.asarray(r["w_out"]) for r in results], axis=0)
    # w_out[b, p, j] corresponds to t = j*SUB + p
    weights = wT.transpose(0, 2, 1).reshape(B, T)[:, :, None]
    return context.astype(np.float32), weights.astype(np.float32)


def kernel(keys, value, query, weight_feedback, v, enc_seq_len):
    from concourse.bass_utils import run_bass_kernel_spmd

    nc = _get_program()
    in_maps = make_in_maps(keys, value, query, weight_feedback, v, enc_seq_len)
    res = run_bass_kernel_spmd(nc, in_maps, core_ids=list(range(NCORES)))
    return assemble_outputs(res.results)


# revision 31
# speedup vs baseline: 1.0103x; 1.0103x over previous
"""AdditiveAttention kernel for 8 TRN2 NeuronCores.

reference:
    hidden  = tanh(keys + query[:,None,:] + weight_feedback)   [B,T,D]
    energies= einsum("btd,d->bt", hidden, v)                   [B,T]
    masked softmax over T (t < enc_seq_len[b])
    context = einsum("bt,btd->bd", weights, value)             [B,D]
    returns (context, weights[:,:,None])

Sharding: data-parallel over B across 8 cores (4 examples per core), no
cross-core communication. DMA-bound: 96 MiB/core must stream from HBM
(~283 us at ~360 GB/s); every compute engine is kept well under that.

Per-core device program, two phases per example b:
  phase A (energies), per 128-row t-subtile j (1MB k/wf DMA tiles on the
  SP HWDGE ring; all small loads + stores ride the ACT ring so SP only
  streams):
    PE  : s_psum = I@k + I@wf + ones_row@q   (fp32r identity-matmul adds,
          full PE rate for moving dim >= 256 — keeps the adds off the DVE)
    ACT : tanh_sb = tanh(s_psum)             (PSUM -> SBUF)
    DVE : e_col = reduce(tanh_sb * v_bcast)  (mul + reduce; TTR would fuse
          them but crashes the device)
    ACT : exp;  DVE: p_col = exp * mask_col
  between phases: S = sum(p) via ones-matmuls + reciprocal; weights
    w = p/S written out (fp32r) — all overlapping phase B's DMAs.
  phase B (context): per value tile, PE accumulates
    ctx_psum[2,D] += w_col.T @ value_tile    (normalized weights, so the
    final tail is only a chunked PSUM->SBUF copy + store).
"""

import numpy as np
from contextlib import ExitStack

B, T, D = 32, 2048, 1024
NCORES = 8
BPC = B // NCORES          # 4 examples per core
SUB = 128                  # t-subtile rows (partition dim)
NT = T // SUB              # 16 subtiles per example
NCH = D // 512             # 512-wide matmul chunks

_PROGRAM_CACHE = {}


def build_program(bpc=BPC, t=T, d=D, wide=1, kb=3, wb=3, vb_bufs=4,
                  two_phase=False, val_engine="sync", val_wide=None):
    import concourse.bacc as bacc
    import concourse.tile as tile
    import concourse.mybir as mybir

    f32 = mybir.dt.float32
    f32r = mybir.dt.float32r
    AF = mybir.ActivationFunctionType
    ALU = mybir.AluOpType
    AX = mybir.AxisListType

    nt = t // SUB
    nch = d // 512
    if val_wide is None:
        val_wide = wide
    assert nt % wide == 0 and nt % val_wide == 0

    nc = bacc.Bacc("TRN2", target_bir_lowering=False, debug=False,
                   num_devices=NCORES, enable_asserts=False)

    keys = nc.dram_tensor("keys", [bpc, t, d], f32r, kind="ExternalInput").ap()
    wf = nc.dram_tensor("wf", [bpc, t, d], f32r, kind="ExternalInput").ap()
    val = nc.dram_tensor("value", [bpc, t, d], f32r, kind="ExternalInput").ap()
    qry = nc.dram_tensor("query", [bpc, d], f32r, kind="ExternalInput").ap()
    vb = nc.dram_tensor("vb", [1, d], f32r, kind="ExternalInput").ap()
    maskT = nc.dram_tensor("maskT", [bpc, SUB, nt], f32, kind="ExternalInput").ap()
    ident = nc.dram_tensor("ident", [SUB, SUB], f32r, kind="ExternalInput").ap()
    ones_row = nc.dram_tensor("ones_row", [1, SUB], f32r, kind="ExternalInput").ap()
    ones_col = nc.dram_tensor("ones_col", [SUB, 1], f32r, kind="ExternalInput").ap()

    ctx_out = nc.dram_tensor("ctx_out", [bpc, d], f32, kind="ExternalOutput").ap()
    w_out = nc.dram_tensor("w_out", [bpc, SUB, nt], f32r, kind="ExternalOutput").ap()

    with tile.TileContext(nc) as tc, ExitStack() as ctx:
        cpool = ctx.enter_context(tc.tile_pool(name="consts", bufs=1))
        kpool = ctx.enter_context(tc.tile_pool(name="kpool", bufs=kb))
        wpool = ctx.enter_context(tc.tile_pool(name="wpool", bufs=wb))
        vpool = ctx.enter_context(tc.tile_pool(name="vpool", bufs=vb_bufs))
        tpool = ctx.enter_context(tc.tile_pool(name="tpool", bufs=2))
        scrpool = ctx.enter_context(tc.tile_pool(name="scrpool", bufs=2))
        smpool = ctx.enter_context(tc.tile_pool(name="smpool", bufs=4))
        expool = ctx.enter_context(tc.tile_pool(name="expool", bufs=2))
        ps_s = ctx.enter_context(tc.tile_pool(name="ps_s", bufs=2, space="PSUM"))
        ps_ctx = ctx.enter_context(tc.tile_pool(name="ps_ctx", bufs=1, space="PSUM"))
        ps_sm = ctx.enter_context(tc.tile_pool(name="ps_sm", bufs=2, space="PSUM"))

        # ident/ones_row feed the very first matmuls — load them first.
        # vb (512KB) and ones_col are needed later; their DMAs are emitted
        # after the first big loads so the DMA engines start on 1MB tiles.
        vb_t = cpool.tile([SUB, d], f32)
        vb_in = cpool.tile([1, d], f32r)
        id_t = cpool.tile([SUB, SUB], f32r)
        nc.scalar.dma_start(out=id_t[:], in_=ident[:])
        onr_t = cpool.tile([1, SUB], f32r)
        nc.scalar.dma_start(out=onr_t[:], in_=ones_row[:])
        onc_t = cpool.tile([SUB, 1], f32r)
        nc.scalar.dma_start(out=vb_in[:], in_=vb[:])
        nc.scalar.dma_start(out=onc_t[:], in_=ones_col[:])

        id_r = id_t[:]
        onr_r = onr_t[:]
        onc_r = onc_t[:]

        # Broadcast v to all partitions on-device (rank-1 ones matmul) —
        # saves a 512KB HBM read on the critical DMA stream.
        vb_ps = ps_ctx.tile([SUB, d], mybir.dt.float32, tag="ctx", name="vb_ps")
        for c in range(nch):
            nc.tensor.matmul(vb_ps[:, 512 * c : 512 * (c + 1)], onr_r,
                             vb_in[0:1, 512 * c : 512 * (c + 1)])
        nc.scalar.copy(vb_t[:], vb_ps[:])

        for b in range(bpc):
            q_t = expool.tile([1, d], f32r, tag="q")
            nc.scalar.dma_start(out=q_t[:], in_=qry[b : b + 1, :])
            mk_t = expool.tile([SUB, nt], f32, tag="mk")
            nc.scalar.dma_start(out=mk_t[:], in_=maskT[b])
            p_t = expool.tile([SUB, nt], f32r, tag="p")
            # fp32r matmul needs even stationary-M and even dst-N: use a
            # 0-stride broadcast of the p column (M=2) and a [2, d] dst.
            ctx_ps = ps_ctx.tile([2, d], f32, tag="ctx")

            def wide_dma(pool, tag, src, jw, engine=nc.sync, w_=None):
                w_ = wide if w_ is None else w_
                tile_ = pool.tile([SUB, w_ * d], f32r, tag=tag, name=tag)
                rows = slice(jw * w_ * SUB, (jw + 1) * w_ * SUB)
                if w_ == 1:
                    engine.dma_start(out=tile_[:], in_=src[b, rows, :])
                else:
                    engine.dma_start(
                        out=tile_[:].rearrange("p (c d) -> p c d", c=w_),
                        in_=src[b, rows, :].rearrange("(c p) d -> p c d",
                                                      c=w_))
                return tile_

            def energies(jw):
                k_t = wide_dma(kpool, "k", keys, jw)
                wf_t = wide_dma(wpool, "wf", wf, jw)
                for u in range(wide):
                    j = jw * wide + u
                    ksub = k_t[:, u * d : (u + 1) * d]
                    wsub = wf_t[:, u * d : (u + 1) * d]

                    s_ps = ps_s.tile([SUB, d], f32, tag="s", name="s_ps")
                    for c in range(nch):
                        sl = slice(512 * c, 512 * (c + 1))
                        nc.tensor.matmul(s_ps[:, sl], id_r, ksub[:, sl],
                                         start=True, stop=False)
                        nc.tensor.matmul(s_ps[:, sl], id_r, wsub[:, sl],
                                         start=False, stop=False)
                        nc.tensor.matmul(s_ps[:, sl], onr_r, q_t[0:1, sl],
                                         start=False, stop=True)

                    th_t = tpool.tile([SUB, d], f32, tag="th", name="th_t")
                    nc.scalar.activation(th_t[:], s_ps[:], AF.Tanh)

                    # (tensor_tensor_reduce would fuse these, but it crashes
                    # the device — NRT_EXEC_UNIT_UNRECOVERABLE)
                    scr_t = scrpool.tile([SUB, d], f32, tag="scr", name="scr_t")
                    e_col = smpool.tile([SUB, 1], f32, tag="ecol", name="e_col")
                    nc.vector.tensor_mul(scr_t[:], th_t[:], vb_t[:])
                    nc.vector.tensor_reduce(e_col[:], scr_t[:], axis=AX.X,
                                            op=ALU.add)

                    pe_col = smpool.tile([SUB, 1], f32, tag="pecol",
                                         name="pe_col")
                    nc.scalar.activation(pe_col[:], e_col[:], AF.Exp)
                    nc.vector.tensor_mul(p_t[:, j : j + 1], pe_col[:],
                                         mk_t[:, j : j + 1])

            def context(jw, v_t, col_src, vw):
                for u in range(vw):
                    j = jw * vw + u
                    p_r = col_src[:, j : j + 1].broadcast_to((SUB, 2))
                    for c in range(nch):
                        nc.tensor.matmul(
                            ctx_ps[:, 512 * c : 512 * (c + 1)], p_r,
                            v_t[:, u * d + 512 * c : u * d + 512 * (c + 1)],
                            start=(j == 0), stop=(j == nt - 1),
                            skip_group_check=True)

            def denom_and_weights():
                """Softmax denominator + weights output. Depends only on p_t
                (end of the energies phase), so it overlaps the value phase.
                Output DMAs ride the ACT HWDGE ring (nc.scalar) so the SP
                ring stays free for input loads (rings are FIFO)."""
                rowsum = smpool.tile([SUB, 1], f32, tag="rowsum")
                nc.vector.tensor_reduce(rowsum[:], p_t[:], axis=AX.X, op=ALU.add)
                rowsum_r = smpool.tile([SUB, 2], f32r, tag="rowsumr")
                nc.vector.tensor_copy(rowsum_r[:],
                                      rowsum[:, 0:1].broadcast_to((SUB, 2)))

                s1_ps = ps_sm.tile([2, 2], f32, tag="sm")
                nc.tensor.matmul(s1_ps[:], onc_r.broadcast_to((SUB, 2)),
                                 rowsum_r[:])
                s1_sb = smpool.tile([1, 2], f32r, tag="s1sb")
                nc.vector.tensor_copy(s1_sb[:], s1_ps[0:1, :])

                sbb_ps = ps_sm.tile([SUB, 2], f32, tag="sm")
                nc.tensor.matmul(sbb_ps[:], onr_r, s1_sb[:])
                S_sb = smpool.tile([SUB, 1], f32, tag="Ssb")
                nc.vector.tensor_copy(S_sb[:], sbb_ps[:, 0:1])
                rS = smpool.tile([SUB, 1], f32, tag="rS")
                nc.vector.reciprocal(rS[:], S_sb[:])

                w_t = expool.tile([SUB, nt], f32r, tag="w")
                nc.vector.tensor_scalar_mul(w_t[:], p_t[:], rS[:])
                nc.scalar.dma_start(out=w_out[b], in_=w_t[:])
                return rS, w_t

            v_eng = nc.sync if val_engine == "sync" else nc.scalar
            ctx_sb = expool.tile([1, d], f32, tag="ctxsb")
            if two_phase:
                for jw in range(nt // wide):
                    energies(jw)
                rS, w_t = denom_and_weights()
                # ctx accumulates already-normalized weights, so after the
                # last value tile only a PSUM->SBUF copy + DMA remain. The
                # last example streams value at fine grain to shorten the
                # post-last-byte critical path.
                vw = val_wide
                for jw in range(nt // vw):
                    if b == bpc - 1 and jw == nt // vw - 1 and vw > 1:
                        # split the very last value tile fine-grained so the
                        # closing ctx matmuls start on a smaller final DMA
                        for j2 in range(vw):
                            ja = jw * vw + j2
                            context(ja, wide_dma(vpool, "v", val, ja,
                                                 engine=v_eng, w_=1), w_t, 1)
                    else:
                        context(jw, wide_dma(vpool, "v", val, jw, engine=v_eng,
                                             w_=vw), w_t, vw)
                for c in range(nch):
                    sl = slice(512 * c, 512 * (c + 1))
                    # alternate engines so the chunk copies run concurrently
                    if c % 2 == 0:
                        nc.vector.tensor_copy(ctx_sb[:, sl], ctx_ps[0:1, sl])
                    else:
                        nc.scalar.copy(ctx_sb[:, sl], ctx_ps[0:1, sl])
                for c in range(nch):
                    sl = slice(512 * c, 512 * (c + 1))
                    nc.scalar.dma_start(out=ctx_out[b : b + 1, sl],
                                        in_=ctx_sb[:, sl])
            else:
                for jw in range(nt // wide):
                    energies(jw)
                    context(jw, wide_dma(vpool, "v", val, jw, engine=v_eng,
                                         w_=wide), p_t, wide)
                rS, w_t = denom_and_weights()
                nc.vector.tensor_scalar_mul(ctx_sb[:], ctx_ps[0:1, :],
                                            rS[0:1, :])
                nc.scalar.dma_start(out=ctx_out[b : b + 1, :], in_=ctx_sb[:])

    nc.compile()
    return nc


BEST = dict(wide=2, kb=4, wb=4, vb_bufs=8, two_phase=True)


def _get_program():
    key = (BPC, T, D)
    if key not in _PROGRAM_CACHE:
        _PROGRAM_CACHE[key] = build_program(*key, **BEST)
    return _PROGRAM_CACHE[key]


def make_in_maps(keys, value, query, weight_feedback, v, enc_seq_len):
    """Shard full inputs into per-core in_maps (+ host-precomputed consts)."""
    keys = np.ascontiguousarray(keys, dtype=np.float32)
    value = np.ascontiguousarray(value, dtype=np.float32)
    query = np.ascontiguousarray(query, dtype=np.float32)
    wf = np.ascontiguousarray(weight_feedback, dtype=np.float32)
    v = np.ascontiguousarray(v, dtype=np.float32)
    lens = np.asarray(enc_seq_len).astype(np.int64)

    vb = v.reshape(1, D).copy()
    ident = np.eye(SUB, dtype=np.float32)
    ones_row = np.ones((1, SUB), dtype=np.float32)
    ones_col = np.ones((SUB, 1), dtype=np.float32)

    # maskT[b, p, j] = 1.0 if (j*SUB + p) < len[b]
    tidx = (np.arange(T).reshape(NT, SUB).T)[None, :, :]      # [1, SUB, NT]
    maskT = (tidx < lens[:, None, None]).astype(np.float32)    # [B, SUB, NT]

    in_maps = []
    for c in range(NCORES):
        s = slice(c * BPC, (c + 1) * BPC)
        in_maps.append({
            "keys": keys[s],
            "wf": wf[s],
            "value": value[s],
            "query": query[s],
            "vb": vb,
            "maskT": np.ascontiguousarray(maskT[s]),
            "ident": ident,
            "ones_row": ones_row,
            "ones_col": ones_col,
        })
    return in_maps


def assemble_outputs(results):
    """results: list (per core) of dicts with ctx_out [BPC,D], w_out [BPC,SUB,NT]."""
    context = np.concatenate([np.asarray(r["ctx_out"]) for r in results], axis=0)
    wT = np.concatenate([np.asarray(r["w_out"]) for r in results], axis=0)
    # w_out[b, p, j] corresponds to t = j*SUB + p
    weights = wT.transpose(0, 2, 1).reshape(B, T)[:, :, None]
    return context.astype(np.float32), weights.astype(np.float32)


def kernel(keys, value, query, weight_feedback, v, enc_seq_len):
    from concourse.bass_utils import run_bass_kernel_spmd

    nc = _get_program()
    in_maps = make_in_maps(keys, value, query, weight_feedback, v, enc_seq_len)
    res = run_bass_kernel_spmd(nc, in_maps, core_ids=list(range(NCORES)))
    return assemble_outputs(res.results)


# revision 33
# speedup vs baseline: 1.5565x; 1.5406x over previous
"""AdditiveAttention kernel for 8 TRN2 NeuronCores.

reference:
    hidden  = tanh(keys + query[:,None,:] + weight_feedback)   [B,T,D]
    energies= einsum("btd,d->bt", hidden, v)                   [B,T]
    masked softmax over T (t < enc_seq_len[b])
    context = einsum("bt,btd->bd", weights, value)             [B,D]
    returns (context, weights[:,:,None])

Sharding: data-parallel over B across 8 cores (4 example slots per core),
no cross-core communication.

Length specialization: rows t >= enc_seq_len[b] are masked to zero weight,
so their keys/weight_feedback/value rows never affect the output and are
never read. kernel() sorts examples by ceil(len/128) and deals them across
cores so every core's slot s holds examples of similar length; the program
is built for the per-slot maximum subtile count (SPMD: one program, all
cores balanced). Remaining bytes ~62% of full for uniform lengths.

Per-core device program, two phases per example slot:
  phase A (energies), per 128-row t-subtile (1MB paired k/wf DMA tiles on
  the SP HWDGE ring; all small loads + stores ride the ACT ring):
    PE  : s_psum = I@k + I@wf + ones_row@q   (fp32r identity-matmul adds,
          full PE rate for moving dim >= 256 — keeps the adds off the DVE)
    ACT : tanh_sb = tanh(s_psum)             (PSUM -> SBUF)
    DVE : e_col = reduce(tanh_sb * v_bcast)  (mul + reduce; TTR would fuse
          them but crashes the device)
    ACT : exp;  DVE: p_col = exp * mask_col
  between phases: S = sum(p) via ones-matmuls + reciprocal; weights
    w = p/S stored (fp32r) — overlapping phase B's DMAs.
  phase B (context): per value tile, PE accumulates
    ctx_psum[2,D] += w_col.T @ value_tile    (normalized weights, so the
    final tail is only a chunked PSUM->SBUF copy + store).
The v-broadcast constant is built on-device from the raw [1,D] vector.
"""

import numpy as np
from contextlib import ExitStack

B, T, D = 32, 2048, 1024
NCORES = 8
BPC = B // NCORES          # 4 example slots per core
SUB = 128                  # t-subtile rows (partition dim)
NT = T // SUB              # max subtiles per example
WIDE = 2                   # subtiles per streaming DMA (1MB)

_PROGRAM_CACHE = {}


def build_program(bpc=BPC, t=T, d=D, slot_nts=None, kb=4, wb=4, vb_bufs=8):
    import concourse.bacc as bacc
    import concourse.tile as tile
    import concourse.mybir as mybir

    f32 = mybir.dt.float32
    f32r = mybir.dt.float32r
    AF = mybir.ActivationFunctionType
    ALU = mybir.AluOpType
    AX = mybir.AxisListType

    nt = t // SUB
    nch = d // 512
    if slot_nts is None:
        slot_nts = (nt,) * bpc
    assert len(slot_nts) == bpc and all(1 <= x <= nt for x in slot_nts)

    nc = bacc.Bacc("TRN2", target_bir_lowering=False, debug=False,
                   num_devices=NCORES, enable_asserts=False)

    keys = nc.dram_tensor("keys", [bpc, t, d], f32r, kind="ExternalInput").ap()
    wf = nc.dram_tensor("wf", [bpc, t, d], f32r, kind="ExternalInput").ap()
    val = nc.dram_tensor("value", [bpc, t, d], f32r, kind="ExternalInput").ap()
    qry = nc.dram_tensor("query", [bpc, d], f32r, kind="ExternalInput").ap()
    vb = nc.dram_tensor("vb", [1, d], f32r, kind="ExternalInput").ap()
    maskT = nc.dram_tensor("maskT", [bpc, SUB, nt], f32, kind="ExternalInput").ap()
    ident = nc.dram_tensor("ident", [SUB, SUB], f32r, kind="ExternalInput").ap()
    ones_row = nc.dram_tensor("ones_row", [1, SUB], f32r, kind="ExternalInput").ap()
    ones_col = nc.dram_tensor("ones_col", [SUB, 1], f32r, kind="ExternalInput").ap()

    ctx_out = nc.dram_tensor("ctx_out", [bpc, d], f32, kind="ExternalOutput").ap()
    w_out = nc.dram_tensor("w_out", [bpc, SUB, nt], f32r, kind="ExternalOutput").ap()

    with tile.TileContext(nc) as tc, ExitStack() as ctx:
        cpool = ctx.enter_context(tc.tile_pool(name="consts", bufs=1))
        kpool = ctx.enter_context(tc.tile_pool(name="kpool", bufs=kb))
        wpool = ctx.enter_context(tc.tile_pool(name="wpool", bufs=wb))
        vpool = ctx.enter_context(tc.tile_pool(name="vpool", bufs=vb_bufs))
        tpool = ctx.enter_context(tc.tile_pool(name="tpool", bufs=2))
        scrpool = ctx.enter_context(tc.tile_pool(name="scrpool", bufs=2))
        smpool = ctx.enter_context(tc.tile_pool(name="smpool", bufs=4))
        expool = ctx.enter_context(tc.tile_pool(name="expool", bufs=2))
        ps_s = ctx.enter_context(tc.tile_pool(name="ps_s", bufs=2, space="PSUM"))
        ps_ctx = ctx.enter_context(tc.tile_pool(name="ps_ctx", bufs=1, space="PSUM"))
        ps_sm = ctx.enter_context(tc.tile_pool(name="ps_sm", bufs=2, space="PSUM"))

        # Small loads ride the ACT HWDGE ring; the SP ring carries only the
        # 1MB streaming loads so it never stalls behind small transfers.
        vb_t = cpool.tile([SUB, d], f32)
        vb_in = cpool.tile([1, d], f32r)
        id_t = cpool.tile([SUB, SUB], f32r)
        nc.scalar.dma_start(out=id_t[:], in_=ident[:])
        onr_t = cpool.tile([1, SUB], f32r)
        nc.scalar.dma_start(out=onr_t[:], in_=ones_row[:])
        onc_t = cpool.tile([SUB, 1], f32r)
        nc.scalar.dma_start(out=vb_in[:], in_=vb[:])
        nc.scalar.dma_start(out=onc_t[:], in_=ones_col[:])

        id_r = id_t[:]
        onr_r = onr_t[:]
        onc_r = onc_t[:]

        # Broadcast v to all partitions on-device (rank-1 ones matmul) —
        # saves a 512KB HBM read on the critical DMA stream.
        vb_ps = ps_ctx.tile([SUB, d], f32, tag="ctx", name="vb_ps")
        for c in range(nch):
            nc.tensor.matmul(vb_ps[:, 512 * c : 512 * (c + 1)], onr_r,
                             vb_in[0:1, 512 * c : 512 * (c + 1)])
        nc.scalar.copy(vb_t[:], vb_ps[:])

        for b in range(bpc):
            ntb = slot_nts[b]
            q_t = expool.tile([1, d], f32r, tag="q")
            nc.scalar.dma_start(out=q_t[:], in_=qry[b : b + 1, :])
            mk_t = expool.tile([SUB, nt], f32, tag="mk")
            nc.scalar.dma_start(out=mk_t[:], in_=maskT[b])
            p_t = expool.tile([SUB, nt], f32r, tag="p")
            # fp32r matmul needs even stationary-M and even dst-N: use a
            # 0-stride broadcast of the w column (M=2) and a [2, d] dst.
            ctx_ps = ps_ctx.tile([2, d], f32, tag="ctx")

            def wide_dma(pool, tag, src, j0, w_, engine=nc.sync):
                tile_ = pool.tile([SUB, w_ * d], f32r, tag=tag, name=tag)
                rows = slice(j0 * SUB, (j0 + w_) * SUB)
                if w_ == 1:
                    engine.dma_start(out=tile_[:], in_=src[b, rows, :])
                else:
                    engine.dma_start(
                        out=tile_[:].rearrange("p (c d) -> p c d", c=w_),
                        in_=src[b, rows, :].rearrange("(c p) d -> p c d",
                                                      c=w_))
                return tile_

            def tile_steps():
                """(start_subtile, width) covering [0, ntb)."""
                steps = [(j0, WIDE) for j0 in range(0, ntb - ntb % WIDE, WIDE)]
                if ntb % WIDE:
                    steps.append((ntb - ntb % WIDE, 1))
                return steps

            def energies(j0, w_):
                k_t = wide_dma(kpool, "k", keys, j0, w_)
                wf_t = wide_dma(wpool, "wf", wf, j0, w_)
                for u in range(w_):
                    j = j0 + u
                    ksub = k_t[:, u * d : (u + 1) * d]
                    wsub = wf_t[:, u * d : (u + 1) * d]

                    s_ps = ps_s.tile([SUB, d], f32, tag="s", name="s_ps")
                    for c in range(nch):
                        sl = slice(512 * c, 512 * (c + 1))
                        nc.tensor.matmul(s_ps[:, sl], id_r, ksub[:, sl],
                                         start=True, stop=False)
                        nc.tensor.matmul(s_ps[:, sl], id_r, wsub[:, sl],
                                         start=False, stop=False)
                        nc.tensor.matmul(s_ps[:, sl], onr_r, q_t[0:1, sl],
                                         start=False, stop=True)

                    th_t = tpool.tile([SUB, d], f32, tag="th", name="th_t")
                    nc.scalar.activation(th_t[:], s_ps[:], AF.Tanh)

                    # (tensor_tensor_reduce would fuse these, but it crashes
                    # the device — NRT_EXEC_UNIT_UNRECOVERABLE)
                    scr_t = scrpool.tile([SUB, d], f32, tag="scr", name="scr_t")
                    e_col = smpool.tile([SUB, 1], f32, tag="ecol", name="e_col")
                    nc.vector.tensor_mul(scr_t[:], th_t[:], vb_t[:])
                    nc.vector.tensor_reduce(e_col[:], scr_t[:], axis=AX.X,
                                            op=ALU.add)

                    pe_col = smpool.tile([SUB, 1], f32, tag="pecol",
                                         name="pe_col")
                    nc.scalar.activation(pe_col[:], e_col[:], AF.Exp)
                    nc.vector.tensor_mul(p_t[:, j : j + 1], pe_col[:],
                                         mk_t[:, j : j + 1])

            def context(j0, w_, v_t, w_src):
                for u in range(w_):
                    j = j0 + u
                    w_r = w_src[:, j : j + 1].broadcast_to((SUB, 2))
                    for c in range(nch):
                        nc.tensor.matmul(
                            ctx_ps[:, 512 * c : 512 * (c + 1)], w_r,
                            v_t[:, u * d + 512 * c : u * d + 512 * (c + 1)],
                            start=(j == 0), stop=(j == ntb - 1),
                            skip_group_check=True)

            def denom_and_weights():
                """Softmax denominator + weights store. Depends only on p_t
                (end of phase A), so it overlaps phase B's DMAs. Stores ride
                the ACT HWDGE ring."""
                rowsum = smpool.tile([SUB, 1], f32, tag="rowsum")
                nc.vector.tensor_reduce(rowsum[:], p_t[:, 0:ntb], axis=AX.X,
                                        op=ALU.add)
                rowsum_r = smpool.tile([SUB, 2], f32r, tag="rowsumr")
                nc.vector.tensor_copy(rowsum_r[:],
                                      rowsum[:, 0:1].broadcast_to((SUB, 2)))

                s1_ps = ps_sm.tile([2, 2], f32, tag="sm")
                nc.tensor.matmul(s1_ps[:], onc_r.broadcast_to((SUB, 2)),
                                 rowsum_r[:])
                s1_sb = smpool.tile([1, 2], f32r, tag="s1sb")
                nc.vector.tensor_copy(s1_sb[:], s1_ps[0:1, :])

                sbb_ps = ps_sm.tile([SUB, 2], f32, tag="sm")
                nc.tensor.matmul(sbb_ps[:], onr_r, s1_sb[:])
                S_sb = smpool.tile([SUB, 1], f32, tag="Ssb")
                nc.vector.tensor_copy(S_sb[:], sbb_ps[:, 0:1])
                rS = smpool.tile([SUB, 1], f32, tag="rS")
                nc.vector.reciprocal(rS[:], S_sb[:])

                w_t = expool.tile([SUB, nt], f32r, tag="w")
                nc.vector.tensor_scalar_mul(w_t[:, 0:ntb], p_t[:, 0:ntb],
                                            rS[:])
                nc.scalar.dma_start(out=w_out[b][:, 0:ntb],
                                    in_=w_t[:, 0:ntb])
                return w_t

            for j0, w_ in tile_steps():
                energies(j0, w_)
            w_t = denom_and_weights()
            # ctx accumulates already-normalized weights, so after the last
            # value tile only a PSUM->SBUF copy + store remain.
            for j0, w_ in tile_steps():
                context(j0, w_, wide_dma(vpool, "v", val, j0, w_), w_t)

            ctx_sb = expool.tile([1, d], f32, tag="ctxsb")
            for c in range(nch):
                sl = slice(512 * c, 512 * (c + 1))
                # alternate engines so the chunk copies run concurrently
                if c % 2 == 0:
                    nc.vector.tensor_copy(ctx_sb[:, sl], ctx_ps[0:1, sl])
                else:
                    nc.scalar.copy(ctx_sb[:, sl], ctx_ps[0:1, sl])
            for c in range(nch):
                sl = slice(512 * c, 512 * (c + 1))
                nc.scalar.dma_start(out=ctx_out[b : b + 1, sl],
                                    in_=ctx_sb[:, sl])

    nc.compile()
    return nc


def plan_slots(enc_seq_len):
    """Sort examples by subtile count and deal across cores so each slot s
    holds similarly-sized examples on every core. Returns (perm, slot_nts):
    core c, slot s runs example perm[s*NCORES + c]; slot_nts[s] is the
    padded (max) subtile count for slot s."""
    lens = np.asarray(enc_seq_len).astype(np.int64)
    n = np.maximum(1, -(-lens // SUB))
    perm = np.argsort(-n, kind="stable")
    slot_nts = tuple(int(n[perm[s * NCORES]]) for s in range(BPC))
    return perm, slot_nts


def _get_program(slot_nts):
    key = (BPC, T, D, slot_nts)
    if key not in _PROGRAM_CACHE:
        _PROGRAM_CACHE[key] = build_program(BPC, T, D, slot_nts=slot_nts)
    return _PROGRAM_CACHE[key]


def make_in_maps(keys, value, query, weight_feedback, v, enc_seq_len, perm):
    keys = np.ascontiguousarray(keys, dtype=np.float32)
    value = np.ascontiguousarray(value, dtype=np.float32)
    query = np.ascontiguousarray(query, dtype=np.float32)
    wf = np.ascontiguousarray(weight_feedback, dtype=np.float32)
    v = np.ascontiguousarray(v, dtype=np.float32)
    lens = np.asarray(enc_seq_len).astype(np.int64)

    vb = v.reshape(1, D).copy()
    ident = np.eye(SUB, dtype=np.float32)
    ones_row = np.ones((1, SUB), dtype=np.float32)
    ones_col = np.ones((SUB, 1), dtype=np.float32)

    # maskT[b, p, j] = 1.0 if (j*SUB + p) < len[b]
    tidx = (np.arange(T).reshape(NT, SUB).T)[None, :, :]      # [1, SUB, NT]
    maskT = (tidx < lens[:, None, None]).astype(np.float32)    # [B, SUB, NT]

    in_maps = []
    for c in range(NCORES):
        idx = [int(perm[s * NCORES + c]) for s in range(BPC)]
        in_maps.append({
            "keys": keys[idx],
            "wf": wf[idx],
            "value": value[idx],
            "query": query[idx],
            "vb": vb,
            "maskT": np.ascontiguousarray(maskT[idx]),
            "ident": ident,
            "ones_row": ones_row,
            "ones_col": ones_col,
        })
    return in_maps


def assemble_outputs(results, perm, slot_nts):
    context = np.zeros((B, D), dtype=np.float32)
    weights = np.zeros((B, T), dtype=np.float32)
    for c in range(NCORES):
        ctx_c = np.asarray(results[c]["ctx_out"])          # [BPC, D]
        wT_c = np.asarray(results[c]["w_out"])             # [BPC, SUB, NT]
        for s in range(BPC):
            bglob = int(perm[s * NCORES + c])
            context[bglob] = ctx_c[s]
            # w_out[s, p, j] corresponds to t = j*SUB + p; only the first
            # slot_nts[s] subtiles are written — the rest is zero (masked).
            ntb = slot_nts[s]
            weights[bglob, : ntb * SUB] = wT_c[s][:, :ntb].T.reshape(ntb * SUB)
    return context, weights[:, :, None]


def kernel(keys, value, query, weight_feedback, v, enc_seq_len):
    from concourse.bass_utils import run_bass_kernel_spmd

    perm, slot_nts = plan_slots(enc_seq_len)
    nc = _get_program(slot_nts)
    in_maps = make_in_maps(keys, value, query, weight_feedback, v,
                           enc_seq_len, perm)
    res = run_bass_kernel_spmd(nc, in_maps, core_ids=list(range(NCORES)))
    return assemble_outputs(res.results, perm, slot_nts)


# revision 37
# speedup vs baseline: 1.5892x; 1.0210x over previous
"""AdditiveAttention kernel for 8 TRN2 NeuronCores.

reference:
    hidden  = tanh(keys + query[:,None,:] + weight_feedback)   [B,T,D]
    energies= einsum("btd,d->bt", hidden, v)                   [B,T]
    masked softmax over T (t < enc_seq_len[b])
    context = einsum("bt,btd->bd", weights, value)             [B,D]
    returns (context, weights[:,:,None])

Sharding: data-parallel over B across 8 cores (4 example slots per core),
no cross-core communication.

Length specialization: rows t >= enc_seq_len[b] are masked to zero weight,
so their keys/weight_feedback/value rows never affect the output and are
never read. kernel() sorts examples by ceil(len/128) and deals them across
cores so every core's slot s holds examples of similar length; the program
is built for the per-slot maximum subtile count (SPMD: one program, all
cores balanced). Remaining bytes ~62% of full for uniform lengths.

Per-core device program, two phases per example slot:
  phase A (energies), per 128-row t-subtile (1MB paired k/wf DMA tiles on
  the SP HWDGE ring; all small loads + stores ride the ACT ring):
    PE  : s_psum = I@k + I@wf + ones_row@q   (fp32r identity-matmul adds,
          full PE rate for moving dim >= 256 — keeps the adds off the DVE)
    ACT : tanh_sb = tanh(s_psum)             (PSUM -> SBUF)
    DVE : e_col = reduce(tanh_sb * v_bcast)  (mul + reduce; TTR would fuse
          them but crashes the device)
    ACT : exp;  DVE: p_col = exp * mask_col
  between phases: S = sum(p) via ones-matmuls + reciprocal; weights
    w = p/S stored (fp32r) — overlapping phase B's DMAs.
  phase B (context): per value tile, PE accumulates
    ctx_psum[2,D] += w_col.T @ value_tile    (normalized weights, so the
    final tail is only a chunked PSUM->SBUF copy + store).
The v-broadcast constant is built on-device from the raw [1,D] vector.
"""

import numpy as np
from contextlib import ExitStack

B, T, D = 32, 2048, 1024
NCORES = 8
BPC = B // NCORES          # 4 example slots per core
SUB = 128                  # t-subtile rows (partition dim)
NT = T // SUB              # max subtiles per example
WIDE = 2                   # subtiles per streaming DMA (1MB)

_PROGRAM_CACHE = {}


def build_program(bpc=BPC, t=T, d=D, slot_nts=None, kb=4, wb=4, vb_bufs=8,
                  order=None):
    import concourse.bacc as bacc
    import concourse.tile as tile
    import concourse.mybir as mybir

    f32 = mybir.dt.float32
    f32r = mybir.dt.float32r
    AF = mybir.ActivationFunctionType
    ALU = mybir.AluOpType
    AX = mybir.AxisListType

    nt = t // SUB
    nch = d // 512
    if slot_nts is None:
        slot_nts = (nt,) * bpc
    assert len(slot_nts) == bpc and all(1 <= x <= nt for x in slot_nts)

    nc = bacc.Bacc("TRN2", target_bir_lowering=False, debug=False,
                   num_devices=NCORES, enable_asserts=False)

    keys = nc.dram_tensor("keys", [bpc, t, d], f32r, kind="ExternalInput").ap()
    wf = nc.dram_tensor("wf", [bpc, t, d], f32r, kind="ExternalInput").ap()
    val = nc.dram_tensor("value", [bpc, t, d], f32r, kind="ExternalInput").ap()
    qry = nc.dram_tensor("query", [bpc, d], f32r, kind="ExternalInput").ap()
    vb = nc.dram_tensor("vb", [1, d], f32r, kind="ExternalInput").ap()
    maskT = nc.dram_tensor("maskT", [bpc, SUB, nt], f32, kind="ExternalInput").ap()
    ident = nc.dram_tensor("ident", [SUB, SUB], f32r, kind="ExternalInput").ap()
    ones_row = nc.dram_tensor("ones_row", [1, SUB], f32r, kind="ExternalInput").ap()
    ones_col = nc.dram_tensor("ones_col", [SUB, 1], f32r, kind="ExternalInput").ap()

    ctx_out = nc.dram_tensor("ctx_out", [bpc, d], f32, kind="ExternalOutput").ap()
    w_out = nc.dram_tensor("w_out", [bpc, SUB, nt], f32r, kind="ExternalOutput").ap()

    with tile.TileContext(nc) as tc, ExitStack() as ctx:
        cpool = ctx.enter_context(tc.tile_pool(name="consts", bufs=1))
        kpool = ctx.enter_context(tc.tile_pool(name="kpool", bufs=kb))
        wpool = ctx.enter_context(tc.tile_pool(name="wpool", bufs=wb))
        vpool = ctx.enter_context(tc.tile_pool(name="vpool", bufs=vb_bufs))
        tpool = ctx.enter_context(tc.tile_pool(name="tpool", bufs=2))
        scrpool = ctx.enter_context(tc.tile_pool(name="scrpool", bufs=2))
        smpool = ctx.enter_context(tc.tile_pool(name="smpool", bufs=4))
        expool = ctx.enter_context(tc.tile_pool(name="expool", bufs=2))
        ps_s = ctx.enter_context(tc.tile_pool(name="ps_s", bufs=2, space="PSUM"))
        ps_ctx = ctx.enter_context(tc.tile_pool(name="ps_ctx", bufs=1, space="PSUM"))
        ps_sm = ctx.enter_context(tc.tile_pool(name="ps_sm", bufs=2, space="PSUM"))

        # Small loads ride the ACT HWDGE ring; the SP ring carries only the
        # 1MB streaming loads so it never stalls behind small transfers.
        vb_t = cpool.tile([SUB, d], f32)
        vb_in = cpool.tile([1, d], f32r)
        id_t = cpool.tile([SUB, SUB], f32r)
        nc.scalar.dma_start(out=id_t[:], in_=ident[:])
        onr_t = cpool.tile([1, SUB], f32r)
        nc.scalar.dma_start(out=onr_t[:], in_=ones_row[:])
        onc_t = cpool.tile([SUB, 1], f32r)
        nc.scalar.dma_start(out=vb_in[:], in_=vb[:])
        nc.scalar.dma_start(out=onc_t[:], in_=ones_col[:])

        id_r = id_t[:]
        onr_r = onr_t[:]
        onc_r = onc_t[:]

        # Broadcast v to all partitions on-device (rank-1 ones matmul) —
        # saves a 512KB HBM read on the critical DMA stream.
        vb_ps = ps_ctx.tile([SUB, d], f32, tag="ctx", name="vb_ps")
        for c in range(nch):
            nc.tensor.matmul(vb_ps[:, 512 * c : 512 * (c + 1)], onr_r,
                             vb_in[0:1, 512 * c : 512 * (c + 1)])
        nc.scalar.copy(vb_t[:], vb_ps[:])

        for b in (order if order is not None else range(bpc)):
            ntb = slot_nts[b]
            q_t = expool.tile([1, d], f32r, tag="q")
            nc.scalar.dma_start(out=q_t[:], in_=qry[b : b + 1, :])
            mk_t = expool.tile([SUB, nt], f32, tag="mk")
            nc.scalar.dma_start(out=mk_t[:], in_=maskT[b])
            p_t = expool.tile([SUB, nt], f32r, tag="p")
            # fp32r matmul needs even stationary-M and even dst-N: use a
            # 0-stride broadcast of the w column (M=2) and a [2, d] dst.
            ctx_ps = ps_ctx.tile([2, d], f32, tag="ctx")

            def wide_dma(pool, tag, src, j0, w_, engine=nc.sync):
                tile_ = pool.tile([SUB, w_ * d], f32r, tag=tag, name=tag)
                rows = slice(j0 * SUB, (j0 + w_) * SUB)
                if w_ == 1:
                    engine.dma_start(out=tile_[:], in_=src[b, rows, :])
                else:
                    engine.dma_start(
                        out=tile_[:].rearrange("p (c d) -> p c d", c=w_),
                        in_=src[b, rows, :].rearrange("(c p) d -> p c d",
                                                      c=w_))
                return tile_

            def tile_steps():
                """(start_subtile, width) covering [0, ntb)."""
                steps = [(j0, WIDE) for j0 in range(0, ntb - ntb % WIDE, WIDE)]
                if ntb % WIDE:
                    steps.append((ntb - ntb % WIDE, 1))
                return steps

            def energies(j0, w_):
                k_t = wide_dma(kpool, "k", keys, j0, w_)
                wf_t = wide_dma(wpool, "wf", wf, j0, w_)
                for u in range(w_):
                    j = j0 + u
                    ksub = k_t[:, u * d : (u + 1) * d]
                    wsub = wf_t[:, u * d : (u + 1) * d]

                    s_ps = ps_s.tile([SUB, d], f32, tag="s", name="s_ps")
                    for c in range(nch):
                        sl = slice(512 * c, 512 * (c + 1))
                        nc.tensor.matmul(s_ps[:, sl], id_r, ksub[:, sl],
                                         start=True, stop=False)
                        nc.tensor.matmul(s_ps[:, sl], id_r, wsub[:, sl],
                                         start=False, stop=False)
                        nc.tensor.matmul(s_ps[:, sl], onr_r, q_t[0:1, sl],
                                         start=False, stop=True)

                    th_t = tpool.tile([SUB, d], f32, tag="th", name="th_t")
                    nc.scalar.activation(th_t[:], s_ps[:], AF.Tanh)

                    # (tensor_tensor_reduce would fuse these, but it crashes
                    # the device — NRT_EXEC_UNIT_UNRECOVERABLE)
                    scr_t = scrpool.tile([SUB, d], f32, tag="scr", name="scr_t")
                    e_col = smpool.tile([SUB, 1], f32, tag="ecol", name="e_col")
                    nc.vector.tensor_mul(scr_t[:], th_t[:], vb_t[:])
                    nc.vector.tensor_reduce(e_col[:], scr_t[:], axis=AX.X,
                                            op=ALU.add)

                    pe_col = smpool.tile([SUB, 1], f32, tag="pecol",
                                         name="pe_col")
                    nc.scalar.activation(pe_col[:], e_col[:], AF.Exp)
                    nc.vector.tensor_mul(p_t[:, j : j + 1], pe_col[:],
                                         mk_t[:, j : j + 1])

            def context(j0, w_, v_t, w_src):
                for u in range(w_):
                    j = j0 + u
                    w_r = w_src[:, j : j + 1].broadcast_to((SUB, 2))
                    for c in range(nch):
                        nc.tensor.matmul(
                            ctx_ps[:, 512 * c : 512 * (c + 1)], w_r,
                            v_t[:, u * d + 512 * c : u * d + 512 * (c + 1)],
                            start=(j == 0), stop=(j == ntb - 1),
                            skip_group_check=True)

            def denom_and_weights():
                """Softmax denominator + weights store. Depends only on p_t
                (end of phase A), so it overlaps phase B's DMAs. Stores ride
                the ACT HWDGE ring."""
                rowsum = smpool.tile([SUB, 1], f32, tag="rowsum")
                nc.vector.tensor_reduce(rowsum[:], p_t[:, 0:ntb], axis=AX.X,
                                        op=ALU.add)
                rowsum_r = smpool.tile([SUB, 2], f32r, tag="rowsumr")
                nc.vector.tensor_copy(rowsum_r[:],
                                      rowsum[:, 0:1].broadcast_to((SUB, 2)))

                s1_ps = ps_sm.tile([2, 2], f32, tag="sm")
                nc.tensor.matmul(s1_ps[:], onc_r.broadcast_to((SUB, 2)),
                                 rowsum_r[:])
                s1_sb = smpool.tile([1, 2], f32r, tag="s1sb")
                nc.vector.tensor_copy(s1_sb[:], s1_ps[0:1, :])

                sbb_ps = ps_sm.tile([SUB, 2], f32, tag="sm")
                nc.tensor.matmul(sbb_ps[:], onr_r, s1_sb[:])
                S_sb = smpool.tile([SUB, 1], f32, tag="Ssb")
                nc.vector.tensor_copy(S_sb[:], sbb_ps[:, 0:1])
                rS = smpool.tile([SUB, 1], f32, tag="rS")
                nc.vector.reciprocal(rS[:], S_sb[:])

                w_t = expool.tile([SUB, nt], f32r, tag="w")
                nc.vector.tensor_scalar_mul(w_t[:, 0:ntb], p_t[:, 0:ntb],
                                            rS[:])
                nc.scalar.dma_start(out=w_out[b][:, 0:ntb],
                                    in_=w_t[:, 0:ntb])
                return w_t

            for j0, w_ in tile_steps():
                energies(j0, w_)
            w_t = denom_and_weights()
            # ctx accumulates already-normalized weights, so after the last
            # value tile only a PSUM->SBUF copy + store remain.
            for j0, w_ in tile_steps():
                context(j0, w_, wide_dma(vpool, "v", val, j0, w_), w_t)

            ctx_sb = expool.tile([1, d], f32, tag="ctxsb")
            for c in range(nch):
                sl = slice(512 * c, 512 * (c + 1))
                # alternate engines so the chunk copies run concurrently
                if c % 2 == 0:
                    nc.vector.tensor_copy(ctx_sb[:, sl], ctx_ps[0:1, sl])
                else:
                    nc.scalar.copy(ctx_sb[:, sl], ctx_ps[0:1, sl])
            for c in range(nch):
                sl = slice(512 * c, 512 * (c + 1))
                nc.scalar.dma_start(out=ctx_out[b : b + 1, sl],
                                    in_=ctx_sb[:, sl])

    nc.compile()
    return nc


def plan_slots(enc_seq_len):
    """Sort examples by subtile count and deal across cores so each slot s
    holds similarly-sized examples on every core. Returns (perm, slot_nts):
    core c, slot s runs example perm[s*NCORES + c]; slot_nts[s] is the
    padded (max) subtile count for slot s."""
    lens = np.asarray(enc_seq_len).astype(np.int64)
    n = np.maximum(1, -(-lens // SUB))
    perm = np.argsort(-n, kind="stable")
    slot_nts = tuple(int(n[perm[s * NCORES]]) for s in range(BPC))
    return perm, slot_nts


def emission_order(slot_nts):
    """Largest slot first (pipeline ramp), second-largest last (its long
    value stream hides the closing denominator chain); middle any order."""
    idx = sorted(range(len(slot_nts)), key=lambda i: -slot_nts[i])
    if len(idx) >= 2:
        return tuple([idx[0]] + idx[2:] + [idx[1]])
    return tuple(idx)


def _get_program(slot_nts):
    key = (BPC, T, D, slot_nts)
    if key not in _PROGRAM_CACHE:
        _PROGRAM_CACHE[key] = build_program(
            BPC, T, D, slot_nts=slot_nts, order=emission_order(slot_nts))
    return _PROGRAM_CACHE[key]


def make_in_maps(keys, value, query, weight_feedback, v, enc_seq_len, perm):
    keys = np.ascontiguousarray(keys, dtype=np.float32)
    value = np.ascontiguousarray(value, dtype=np.float32)
    query = np.ascontiguousarray(query, dtype=np.float32)
    wf = np.ascontiguousarray(weight_feedback, dtype=np.float32)
    v = np.ascontiguousarray(v, dtype=np.float32)
    lens = np.asarray(enc_seq_len).astype(np.int64)

    vb = v.reshape(1, D).copy()
    ident = np.eye(SUB, dtype=np.float32)
    ones_row = np.ones((1, SUB), dtype=np.float32)
    ones_col = np.ones((SUB, 1), dtype=np.float32)

    # maskT[b, p, j] = 1.0 if (j*SUB + p) < len[b]
    tidx = (np.arange(T).reshape(NT, SUB).T)[None, :, :]      # [1, SUB, NT]
    maskT = (tidx < lens[:, None, None]).astype(np.float32)    # [B, SUB, NT]

    in_maps = []
    for c in range(NCORES):
        idx = [int(perm[s * NCORES + c]) for s in range(BPC)]
        in_maps.append({
            "keys": keys[idx],
            "wf": wf[idx],
            "value": value[idx],
            "query": query[idx],
            "vb": vb,
            "maskT": np.ascontiguousarray(maskT[idx]),
            "ident": ident,
            "ones_row": ones_row,
            "ones_col": ones_col,
        })
    return in_maps


def assemble_outputs(results, perm, slot_nts):
    context = np.zeros((B, D), dtype=np.float32)
    weights = np.zeros((B, T), dtype=np.float32)
    for c in range(NCORES):
        ctx_c = np.asarray(results[c]["ctx_out"])          # [BPC, D]
        wT_c = np.asarray(results[c]["w_out"])             # [BPC, SUB, NT]
        for s in range(BPC):
            bglob = int(perm[s * NCORES + c])
            context[bglob] = ctx_c[s]
            # w_out[s, p, j] corresponds to t = j*SUB + p; only the first
            # slot_nts[s] subtiles are written — the rest is zero (masked).
            ntb = slot_nts[s]
            weights[bglob, : ntb * SUB] = wT_c[s][:, :ntb].T.reshape(ntb * SUB)
    return context, weights[:, :, None]


def kernel(keys, value, query, weight_feedback, v, enc_seq_len):
    from concourse.bass_utils import run_bass_kernel_spmd

    perm, slot_nts = plan_slots(enc_seq_len)
    nc = _get_program(slot_nts)
    in_maps = make_in_maps(keys, value, query, weight_feedback, v,
                           enc_seq_len, perm)
    res = run_bass_kernel_spmd(nc, in_maps, core_ids=list(range(NCORES)))
    return assemble_outputs(res.results, perm, slot_nts)
